# revision 23
# baseline (speedup 1.0000x reference)
"""MoE layer (8 experts, top-2, capacity 2560) on 8 Trainium2 NeuronCores.

Strategy (expert-parallel, per the sharding hint):
  - Router is token-sharded: core r computes logits/top-2/softmax weights for
    tokens [1024r, 1024r+1024); a small AllGather shares the routing metadata.
  - Every core then (replicated) computes capacity positions for all 16384
    assignments with prefix-sums (tensor_tensor_scan + triangular matmul),
    compacts its own expert's token list with GPSIMD sparse_gather, and
    gathers its <=2560 token rows via indirect DMA.
  - Expert FFN gelu(x@w1[e])@w2[e] runs as fp32r matmuls (full PE rate).
  - The per-expert outputs are AllGathered; each core combines (weighted
     2-way sum via indirect row gather) the output rows for its token slice.

kernel(**inputs) takes FULL inputs and returns the FULL (8192, 1024) output.
"""
import os
import numpy as np

import concourse.bass as bass
import concourse.bacc as bacc
import concourse.tile as tile
from concourse import mybir
from concourse.bass_utils import run_bass_kernel_spmd
from concourse.masks import make_identity

F32 = mybir.dt.float32
F32R = mybir.dt.float32r
BF16 = mybir.dt.bfloat16
I32 = mybir.dt.int32
U32 = mybir.dt.uint32
AL = mybir.AluOpType
ACT = mybir.ActivationFunctionType

E = 8            # experts == cores
N = 8192         # tokens
H = 1024         # hidden
II = 4096        # intermediate
C = 2560         # per-expert capacity
NS = N // E      # tokens per core slice (1024)
CHALF = C // 2   # 1280, FFN processed in two C-halves
P = 128
NCHUNK = 8       # compaction chunks over the 16384-assignment stream
CPAD = C // NCHUNK  # 320 padded slots per (chunk, expert)

DEBUG = bool(int(os.environ.get("MOE_DEBUG", "0")))
# FFN matmul dtype: "f32r" (fp32 data, fp32r matmuls) or "bf16"
FFN_MODE = os.environ.get("MOE_FFN", "f32r")
# dtype of the y buffer exchanged by the big AllGather
AG_BF16 = bool(int(os.environ.get("MOE_AG_BF16", "0")))
# gelu path: "hw_tanh" = 1-op ACT table Gelu_apprx_tanh; "sigmoid5" =
# explicit exact tanh-gelu via Sigmoid (sim-supported, more engine ops)
GELU_MODE = os.environ.get("MOE_GELU", "hw_tanh")

_CACHED = {}


def _build_nc():
    nc = bacc.Bacc("TRN2", target_bir_lowering=False, debug=False,
                   num_devices=E)

    ffn_bf16_ = FFN_MODE == "bf16"
    wdt = F32 if ffn_bf16_ else F32R
    tokens = nc.dram_tensor("tokens", [N, H], wdt, kind="ExternalInput").ap()
    my_tokens = nc.dram_tensor("my_tokens", [NS, H], F32,
                               kind="ExternalInput").ap()
    rw = nc.dram_tensor("rw", [H, E], F32, kind="ExternalInput").ap()
    w1s = nc.dram_tensor("w1s", [H, II], wdt, kind="ExternalInput").ap()
    w2s = nc.dram_tensor("w2s", [II, H], wdt, kind="ExternalInput").ap()
    eid128 = nc.dram_tensor("eid128", [P, 1], F32, kind="ExternalInput").ap()
    # host constants for the chunked compaction (j-stream chunk = 16
    # partitions = 2048 assignments; 320 padded slots per chunk/expert)
    ltri_blk = nc.dram_tensor("ltri_blk", [P, P], F32,
                              kind="ExternalInput").ap()
    cb320 = nc.dram_tensor("cb320", [P, 1], F32, kind="ExternalInput").ap()
    out_slice = nc.dram_tensor("out_slice", [NS, H], F32,
                               kind="ExternalOutput").ap()
    dbg = {}
    if DEBUG:
        for nm, shp, dt in [("d_meta", [N * 4], F32),
                            ("d_pa", [P, P], F32),
                            ("d_gidx", [P, 20], I32),
                            ("d_sl1", [P, 64], F32),
                            ("d_we1", [P, 8], F32),
                            ("d_nf", [1, 1], U32)]:
            dbg[nm] = nc.dram_tensor(nm, shp, dt, kind="ExternalOutput").ap()

    ffn_bf16 = FFN_MODE == "bf16"
    ffn_dt = BF16 if ffn_bf16 else F32R
    mm_dt = BF16 if ffn_bf16 else F32R
    y_dt = BF16 if AG_BF16 else F32

    with tile.TileContext(nc) as tc:
        _body(nc, tc, tokens, my_tokens, rw, w1s, w2s, eid128, ltri_blk,
              cb320, out_slice, dbg, ffn_dt, mm_dt, y_dt)
    nc.compile()
    return nc


def _body(nc, tc, tokens, my_tokens, rw, w1s, w2s, eid128, ltri_blk_d,
          cb320_d, out_slice, dbg, ffn_dt, mm_dt, y_dt):
    with tc.tile_pool(name="persist", bufs=1) as pp, \
         tc.tile_pool(name="pdram", bufs=1, space="DRAM") as pd:
        # ---------- constants ----------
        ident = pp.tile([P, P], F32)
        make_identity(nc, ident[:])

        # strict lower-in-k triangular: L[k, m] = 1 iff k < m
        ltri = pp.tile([P, P], F32)
        nc.gpsimd.memset(ltri[:], 0.0)
        nc.gpsimd.affine_select(out=ltri[:], in_=ltri[:],
                                compare_op=AL.is_ge, fill=1.0, base=0,
                                pattern=[[-1, P]], channel_multiplier=1)
        # token-id + 1 in assignment-stream layout j = 128*p + f
        tidp1 = pp.tile([P, 64, 2], F32)
        nc.gpsimd.iota(tidp1[:], pattern=[[1, 64], [0, 2]], base=1,
                       channel_multiplier=64,
                       allow_small_or_imprecise_dtypes=True)
        rw_sb = pp.tile([P, E, E], F32)  # [p, hc, e]
        nc.sync.dma_start(rw_sb[:], rw.rearrange("(hc p) e -> p hc e", p=P))
        eid_sb = pp.tile([P, 1], F32)
        nc.sync.dma_start(eid_sb[:], eid128[:])
        ltrib = pp.tile([P, P], F32)
        nc.sync.dma_start(ltrib[:], ltri_blk_d[:])
        cb_sb = pp.tile([P, 1], F32)
        nc.sync.dma_start(cb_sb[:], cb320_d[:])

        # persistent small results
        gidx = pp.tile([P, C // P], I32)      # dispatch gather indices
        si1 = pp.tile([P, E], I32)            # my-slice slot indices (k=0)
        si2 = pp.tile([P, E], I32)
        we1 = pp.tile([P, E], F32)            # my-slice effective weights
        we2 = pp.tile([P, E], F32)

        # DRAM scratch
        meta_in = pd.tile([NS * 4], F32)
        meta_all = pd.tile([N * 4], F32)
        estream = pd.tile([2 * N], F32)
        vstream = pd.tile([2 * N], F32)
        pkstream = pd.tile([2 * N], F32)
        idxbuf = pd.tile([C], I32)
        y_in = pd.tile([C, H], y_dt)
        y_all = pd.tile([E * C, H], y_dt, addr_space="Shared")

        # ================= P1: router on my token slice =================
        with tc.tile_pool(name="router", bufs=1) as rp, \
             tc.tile_pool(name="rps", bufs=2, space="PSUM") as rps:
            mtT = rp.tile([P, E, NS], F32)   # [h, hc, t] transposed slice
            for ti in range(E):
                mt_t = rp.tile([P, H], F32, tag="mt_t", bufs=2)
                nc.sync.dma_start(mt_t[:], my_tokens[P * ti:P * (ti + 1), :])
                for hc in range(E):
                    tp = rps.tile([P, P], F32, tag="tp")
                    nc.tensor.transpose(tp[:], mt_t[:, P * hc:P * (hc + 1)],
                                        ident[:])
                    nc.scalar.copy(mtT[:, hc, P * ti:P * (ti + 1)], tp[:])

            lgt = rp.tile([P, E, E], F32)    # [t, ti, e] logits
            for ti in range(E):
                ps_l = rps.tile([P, E], F32, tag="ps_l")
                for hc in range(E):
                    nc.tensor.matmul(ps_l[:], mtT[:, hc, P * ti:P * (ti + 1)],
                                     rw_sb[:, hc, :], start=(hc == 0),
                                     stop=(hc == E - 1))
                nc.vector.tensor_copy(lgt[:, ti, :], ps_l[:])

            # softmax pieces + top-2
            nmax = rp.tile([P, E], F32)
            nc.vector.tensor_reduce(nmax[:], lgt[:], mybir.AxisListType.X,
                                    AL.max, negate=True)
            zsum = rp.tile([P, E], F32)
            exps = rp.tile([P, E], F32)
            meta_sb = rp.tile([P, 4 * E], F32)
            m8v = rp.tile([P, E], F32)
            m8i = rp.tile([P, E], U32)
            e2 = rp.tile([P, 1], F32, tag="e2", bufs=2)
            den = rp.tile([P, 1], F32, tag="den", bufs=2)
            for ti in range(E):
                nc.scalar.activation(exps[:], lgt[:, ti, :], ACT.Exp,
                                     bias=nmax[:, ti:ti + 1], scale=1.0,
                                     accum_out=zsum[:, ti:ti + 1])
                nc.vector.max_with_indices(m8v[:], m8i[:], lgt[:, ti, :])
                nc.vector.tensor_copy(meta_sb[:, ti:ti + 1], m8i[:, 0:1])
                nc.vector.tensor_copy(meta_sb[:, E + ti:E + ti + 1],
                                      m8i[:, 1:2])
                e2_ = e2  # exp(v2 - m1)
                nc.scalar.activation(e2_[:], m8v[:, 1:2], ACT.Exp,
                                     bias=nmax[:, ti:ti + 1], scale=1.0)
                # den = (Z*eps + 1) + e2 ; w1 = 1/den ; w2 = e2/den
                nc.vector.tensor_scalar(den[:], zsum[:, ti:ti + 1], 1e-6, 1.0,
                                        AL.mult, AL.add)
                nc.vector.tensor_tensor(den[:], den[:], e2_[:], AL.add)
                nc.vector.reciprocal(meta_sb[:, 2 * E + ti:2 * E + ti + 1],
                                     den[:])
                nc.vector.tensor_tensor(
                    meta_sb[:, 3 * E + ti:3 * E + ti + 1],
                    meta_sb[:, 2 * E + ti:2 * E + ti + 1], e2_[:], AL.mult)
            nc.sync.dma_start(meta_in[:].rearrange("(f p) -> p f", p=P),
                              meta_sb[:])

        # ================= P2: AllGather routing metadata =================
        nc.gpsimd.collective_compute(
            "AllGather", AL.bypass, replica_groups=[list(range(E))],
            ins=[meta_in[:]], outs=[meta_all[:]])
        if DEBUG:
            nc.sync.dma_start(dbg["d_meta"][:], meta_all[:])

        # meta_all[r*4096 + fld*1024 + ti*128 + p] = field fld of token
        # t = 1024r + 128ti + p.  t-layout view (p, g) with g = 8r + ti.
        meta_v = meta_all[:].rearrange("(r fld ti p) -> p r ti fld",
                                       fld=4, ti=E, p=P)
        # estream[2t + k] = expert id of assignment (t, k)
        # (DMA APs are limited to 3 dims -> one DMA per metadata rank block)
        es_v = estream[:].rearrange("(r ti p k) -> p r ti k", p=P, ti=E, k=2)
        for r in range(E):
            for k in range(2):
                nc.gpsimd.dma_start(es_v[:, r, :, k], meta_v[:, r, :, k])

        # ================= P4: prefix sums, slots, dispatch list ==========
        with tc.tile_pool(name="route2", bufs=1) as qp, \
             tc.tile_pool(name="qps", bufs=2, space="PSUM") as qps:
            ea = qp.tile([P, P], F32)
            nc.sync.dma_start(ea[:], estream[:].rearrange("(p f) -> p f", p=P))

            t8 = qp.tile([P, E], F32)
            masks = []
            for e in range(E):
                m_e = qp.tile([P, P], F32, name=f"mask{e}")
                nc.vector.tensor_scalar(m_e[:], ea[:], float(e), None,
                                        AL.is_equal)
                masks.append(m_e)
            scans = []
            for e in range(E):
                s_e = qp.tile([P, P], F32, name=f"scan{e}")
                nc.vector.tensor_tensor_scan(s_e[:], masks[e][:], masks[e][:],
                                             0.0, AL.add, AL.bypass)
                nc.vector.tensor_copy(t8[:, e:e + 1], s_e[:, P - 1:P])
                scans.append(s_e)
            # global offsets (for exact capacity keep) and block-local
            # offsets (for the chunked slot layout)
            offp = qps.tile([P, E], F32)
            nc.tensor.matmul(offp[:], ltri[:], t8[:], start=True, stop=True)
            offm1 = qp.tile([P, E], F32)
            nc.vector.tensor_scalar(offm1[:], offp[:], 1.0, None, AL.subtract)
            offbp = qps.tile([P, E], F32)
            nc.tensor.matmul(offbp[:], ltrib[:], t8[:], start=True, stop=True)
            offbm1 = qp.tile([P, E], F32)
            nc.vector.tensor_scalar(offbm1[:], offbp[:], 1.0, None,
                                    AL.subtract)

            pag = qp.tile([P, P], F32)   # exclusive pos, global per expert
            pal = qp.tile([P, P], F32)   # exclusive pos within 2048-chunk
            tmp = qp.tile([P, P], F32)
            for e in range(E):
                s_e = scans[e]
                nc.vector.tensor_scalar_add(tmp[:], s_e[:],
                                            offbm1[:, e:e + 1])
                nc.vector.tensor_tensor(tmp[:], tmp[:], masks[e][:], AL.mult)
                if e == 0:
                    nc.vector.tensor_copy(pal[:], tmp[:])
                else:
                    nc.vector.tensor_tensor(pal[:], pal[:], tmp[:], AL.add)
                nc.vector.tensor_scalar_add(s_e[:], s_e[:], offm1[:, e:e + 1])
                nc.vector.tensor_tensor(s_e[:], s_e[:], masks[e][:], AL.mult)
                if e == 0:
                    nc.vector.tensor_copy(pag[:], s_e[:])
                else:
                    nc.vector.tensor_tensor(pag[:], pag[:], s_e[:], AL.add)
            if DEBUG:
                nc.sync.dma_start(dbg["d_pa"][:], pal[:])

            kp = qp.tile([P, P], F32)
            nc.vector.tensor_scalar(kp[:], pag[:], float(C), None, AL.is_lt)
            nc.vector.tensor_scalar(tmp[:], pal[:], float(CPAD), None,
                                    AL.is_lt)
            nc.vector.tensor_tensor(kp[:], kp[:], tmp[:], AL.mult)
            # pk = keep*(e*C + chunk_base + lpos + 1) - 1
            slt = qp.tile([P, P], F32)
            nc.vector.tensor_scalar(slt[:], ea[:], float(C), 1.0,
                                    AL.mult, AL.add)
            nc.vector.tensor_tensor(slt[:], slt[:], pal[:], AL.add)
            nc.vector.tensor_scalar_add(slt[:], slt[:], cb_sb[:, 0:1])
            nc.vector.tensor_tensor(slt[:], slt[:], kp[:], AL.mult)
            nc.vector.tensor_scalar_add(slt[:], slt[:], -1.0)
            nc.sync.dma_start(pkstream[:].rearrange("(p f) -> p f", p=P),
                              slt[:])

            # dispatch value stream: mine&keep ? tid : -1
            mm = qp.tile([P, P], F32)
            nc.vector.tensor_scalar(mm[:], ea[:], eid_sb[:, 0:1], None,
                                    AL.is_equal)
            nc.vector.tensor_tensor(mm[:], mm[:], kp[:], AL.mult)
            nc.vector.tensor_tensor(
                mm[:], mm[:], tidp1[:].rearrange("p a b -> p (a b)"), AL.mult)
            nc.vector.tensor_scalar_add(mm[:], mm[:], -1.0)
            nc.sync.dma_start(vstream[:].rearrange("(p f) -> p f", p=P), mm[:])

            # chunked compaction: 8 x (2048 assignments -> 320 padded slots)
            for c in range(NCHUNK):
                sg_in = qp.tile([16, 2 * N // NCHUNK // 16], F32,
                                tag="sg_in", bufs=2)
                nc.sync.dma_start(
                    sg_in[:], vstream[:].rearrange("(c f p) -> c p f",
                                                   c=NCHUNK, p=16)[c])
                sg_out = qp.tile([16, CPAD // 16], F32, tag="sg_out", bufs=2)
                nf = qp.tile([1, 1], U32, tag="nf", bufs=2)
                nc.gpsimd.sparse_gather(sg_out[:], sg_in[:], num_found=nf[:])
                if DEBUG and c == 0:
                    nc.sync.dma_start(dbg["d_nf"][:], nf[:])
                # -1 padding -> large (skipped by bounds check)
                pad = qp.tile([16, CPAD // 16], F32, tag="pad", bufs=2)
                nc.vector.tensor_scalar(pad[:], sg_out[:], 0.0, 30000.0,
                                        AL.is_lt, AL.mult)
                nc.vector.tensor_tensor(sg_out[:], sg_out[:], pad[:], AL.add)
                sgi = qp.tile([16, CPAD // 16], I32, tag="sgi", bufs=2)
                nc.vector.tensor_copy(sgi[:], sg_out[:])
                nc.sync.dma_start(
                    idxbuf[:].rearrange("(c f p) -> c p f",
                                        c=NCHUNK, p=16)[c], sgi[:])
            nc.sync.dma_start(gidx[:], idxbuf[:].rearrange("(f p) -> p f",
                                                           p=P))
            if DEBUG:
                nc.sync.dma_start(dbg["d_gidx"][:], gidx[:])

            # per-token slots/weights for the combine, then my slice
            pk_v = pkstream[:].rearrange("(g p k) -> p g k", p=P, k=2)
            pid = nc.vector.partition_id()
            for k, (si_t, we_t, fld) in enumerate([(si1, we1, 2),
                                                   (si2, we2, 3)]):
                slk = qp.tile([P, 64], F32, name=f"slk{k}")
                nc.sync.dma_start(slk[:], pk_v[:, :, k])
                wmask = qp.tile([P, 64], F32, name=f"wmask{k}")
                nc.vector.tensor_scalar(wmask[:], slk[:], 0.0, None, AL.is_ge)
                wall = qp.tile([P, 64], F32, name=f"wall{k}")
                for r in range(E):
                    nc.sync.dma_start(wall[:, E * r:E * (r + 1)],
                                      meta_v[:, r, :, fld])
                nc.vector.tensor_tensor(wall[:], wall[:], wmask[:], AL.mult)
                nc.vector.tensor_scalar_max(slk[:], slk[:], 0.0)
                slki = qp.tile([P, 64], I32, name=f"slki{k}")
                nc.vector.tensor_copy(slki[:], slk[:])
                if DEBUG and k == 0:
                    nc.sync.dma_start(dbg["d_sl1"][:], slk[:])
                nc.vector.tensor_copy(si_t[:], slki[:, bass.ts(pid, E)])
                nc.vector.tensor_copy(we_t[:], wall[:, bass.ts(pid, E)])
            if DEBUG:
                nc.sync.dma_start(dbg["d_we1"][:], we1[:])

        # ================= P5-P7: dispatch gather + FFN per C-half ========
        IGN = 8                    # I-groups of 512
        IGW = II // IGN            # 512
        CBS = [(0, 512), (512, 512), (1024, 256)]   # c-blocks within a half
        for hf in range(2):
            with tc.tile_pool(name=f"ffn{hf}", bufs=1) as fp, \
                 tc.tile_pool(name=f"fps{hf}", bufs=2, space="PSUM") as fps, \
                 tc.tile_pool(name=f"fps2{hf}", bufs=3, space="PSUM") as fps2:
                xT = fp.tile([P, E, CHALF], ffn_dt)     # [h, hc, c]
                for ct in range(CHALF // P):
                    xg = fp.tile([P, H], F32, tag="xg", bufs=3)
                    nc.gpsimd.indirect_dma_start(
                        out=xg[:], out_offset=None,
                        in_=tokens[:].bitcast(F32),
                        in_offset=bass.IndirectOffsetOnAxis(
                            ap=gidx[:, hf * 10 + ct:hf * 10 + ct + 1],
                            axis=0),
                        bounds_check=N - 1, oob_is_err=False)
                    for hc in range(E):
                        tp2 = fps.tile([P, P], F32, tag="tp2")
                        nc.tensor.transpose(tp2[:],
                                            xg[:, P * hc:P * (hc + 1)],
                                            ident[:])
                        nc.scalar.copy(xT[:, hc, P * ct:P * (ct + 1)], tp2[:])

                y_sb = fp.tile([P, CHALF // P, H], F32)  # [c, ct, h]
                for ig in range(IGN):
                    w1g = fp.tile([P, E, IGW], ffn_dt, tag="w1g", bufs=2)
                    nc.gpsimd.dma_start(
                        w1g[:], w1s.rearrange("(hc p) i -> p hc i",
                                              p=P)[:, :, IGW * ig:IGW * (ig + 1)])
                    w2g = fp.tile([P, IGW // P, H], ffn_dt, tag="w2g", bufs=2)
                    nc.gpsimd.dma_start(
                        w2g[:], w2s.rearrange("(ic p) h -> p ic h",
                                              p=P)[:, 4 * ig:4 * (ig + 1), :])
                    for (c0, cw) in CBS:
                        g_sb = fp.tile([P, IGW // P, 512], ffn_dt,
                                       tag="g_sb", bufs=2)
                        for iic in range(IGW // P):
                            hps = fps.tile([P, 512], F32, tag="hps")
                            for hc in range(E):
                                nc.tensor.matmul(
                                    hps[:, :cw],
                                    w1g[:, hc, P * iic:P * (iic + 1)].bitcast(mm_dt),
                                    xT[:, hc, c0:c0 + cw].bitcast(mm_dt),
                                    start=(hc == 0), stop=(hc == E - 1))
                            if GELU_MODE == "hw_tanh":
                                nc.scalar.activation(g_sb[:, iic, :cw],
                                                     hps[:, :cw],
                                                     ACT.Gelu_apprx_tanh)
                            else:
                                # exact tanh-gelu: x*sigmoid(1.5957691*
                                #   (x + 0.044715*x^3))
                                xc = fp.tile([P, 512], F32, tag="gxc",
                                             bufs=2)
                                nc.scalar.copy(xc[:, :cw], hps[:, :cw])
                                gt = fp.tile([P, 512], F32, tag="ggt",
                                             bufs=2)
                                nc.vector.tensor_tensor(gt[:, :cw],
                                                        xc[:, :cw],
                                                        xc[:, :cw], AL.mult)
                                nc.vector.tensor_scalar(gt[:, :cw],
                                                        gt[:, :cw],
                                                        0.044715, 1.0,
                                                        AL.mult, AL.add)
                                nc.vector.tensor_tensor(gt[:, :cw],
                                                        gt[:, :cw],
                                                        xc[:, :cw], AL.mult)
                                nc.scalar.activation(gt[:, :cw], gt[:, :cw],
                                                     ACT.Sigmoid,
                                                     scale=1.5957691216057308)
                                nc.vector.tensor_tensor(g_sb[:, iic, :cw],
                                                        gt[:, :cw],
                                                        xc[:, :cw], AL.mult)
                        for cc in range(cw // P):
                            ct = (c0 + P * cc) // P
                            for hh in range(2):
                                yps = fps2.tile([P, 512], F32, tag="yps")
                                for iic in range(IGW // P):
                                    nc.tensor.matmul(
                                        yps[:],
                                        g_sb[:, iic, P * cc:P * (cc + 1)].bitcast(mm_dt),
                                        w2g[:, iic, 512 * hh:512 * (hh + 1)].bitcast(mm_dt),
                                        start=(iic == 0),
                                        stop=(iic == IGW // P - 1))
                                ysl = y_sb[:, ct, 512 * hh:512 * (hh + 1)]
                                if ig == 0:
                                    nc.vector.tensor_copy(ysl, yps[:])
                                else:
                                    nc.vector.tensor_tensor(ysl, ysl, yps[:],
                                                            AL.add)
                # evict this half's y to DRAM (AG input)
                for ct in range(CHALF // P):
                    if y_dt == F32:
                        nc.sync.dma_start(
                            y_in[hf * CHALF + P * ct:hf * CHALF + P * (ct + 1), :],
                            y_sb[:, ct, :])
                    else:
                        yb = fp.tile([P, H], y_dt, tag="yb", bufs=2)
                        nc.vector.tensor_copy(yb[:], y_sb[:, ct, :])
                        nc.sync.dma_start(
                            y_in[hf * CHALF + P * ct:hf * CHALF + P * (ct + 1), :],
                            yb[:])

        # ================= P8: AllGather expert outputs ===================
        nc.gpsimd.collective_compute(
            "AllGather", AL.bypass, replica_groups=[list(range(E))],
            ins=[y_in[:]], outs=[y_all[:]])

        # ================= P9: combine my token slice =====================
        with tc.tile_pool(name="comb", bufs=1) as cp:
            for ti in range(E):
                g1 = cp.tile([P, H], y_dt, tag="g1", bufs=2)
                g2 = cp.tile([P, H], y_dt, tag="g2", bufs=2)
                nc.gpsimd.indirect_dma_start(
                    out=g1[:], out_offset=None, in_=y_all[:],
                    in_offset=bass.IndirectOffsetOnAxis(
                        ap=si1[:, ti:ti + 1], axis=0))
                nc.gpsimd.indirect_dma_start(
                    out=g2[:], out_offset=None, in_=y_all[:],
                    in_offset=bass.IndirectOffsetOnAxis(
                        ap=si2[:, ti:ti + 1], axis=0))
                o1 = cp.tile([P, H], F32, tag="o1", bufs=2)
                o2 = cp.tile([P, H], F32, tag="o2", bufs=2)
                nc.vector.tensor_scalar_mul(o1[:], g1[:], we1[:, ti:ti + 1])
                nc.vector.tensor_scalar_mul(o2[:], g2[:], we2[:, ti:ti + 1])
                nc.vector.tensor_tensor(o1[:], o1[:], o2[:], AL.add)
                nc.sync.dma_start(out_slice[P * ti:P * (ti + 1), :], o1[:])


def _get_nc():
    key = (FFN_MODE, AG_BF16, DEBUG)
    if key not in _CACHED:
        _CACHED[key] = _build_nc()
    return _CACHED[key]


def kernel(tokens, router_weight, w1, w2):
    tokens = np.ascontiguousarray(np.asarray(tokens, dtype=np.float32))
    router_weight = np.ascontiguousarray(np.asarray(router_weight,
                                                    dtype=np.float32))
    w1 = np.asarray(w1, dtype=np.float32)
    w2 = np.asarray(w2, dtype=np.float32)

    nc = _get_nc()
    k = np.arange(P)
    ltri_blk = ((k[:, None] < k[None, :]) &
                (k[:, None] // 16 == k[None, :] // 16)).astype(np.float32)
    cb320 = ((k // 16) * CPAD).astype(np.float32).reshape(P, 1)
    in_maps = []
    for r in range(E):
        in_maps.append({
            "tokens": tokens,
            "my_tokens": np.ascontiguousarray(tokens[NS * r:NS * (r + 1)]),
            "rw": router_weight,
            "w1s": np.ascontiguousarray(w1[r]),
            "w2s": np.ascontiguousarray(w2[r]),
            "eid128": np.full((P, 1), float(r), np.float32),
            "ltri_blk": ltri_blk,
            "cb320": cb320,
        })
    trace = bool(int(os.environ.get("MOE_TRACE", "0")))
    if trace:
        try:
            import antenv.axon_hooks  # noqa: F401
        except ImportError:
            trace = False
    res = run_bass_kernel_spmd(nc, in_maps, core_ids=list(range(E)),
                               trace=trace)
    kernel.last_results = res
    out = np.concatenate([res.results[r]["out_slice"] for r in range(E)],
                         axis=0)
    return out
